# revision 18
# baseline (speedup 1.0000x reference)
"""Trainium2 Bass kernel for nn_AnotherDDoIGRUCell.

Math (per timestep, parallel part folded on host):
  x_tm1 = x_{t-1} @ M,  x_tm2 = x_{t-2} @ M @ M   with M = C.T @ C
  d_x   = x_t - x_tm1 ; dd_x = x_t - 2*x_tm1 + x_tm2
  r = sigmoid(x_t@Wxr + d_x@dWxr + dd_x@ddWxr + h@Whr + br)
  u = sigmoid(x_t@Wxu + d_x@dWxu + dd_x@ddWxu + h@Whu + bu)
  c = tanh(x_t@Wxh + (r*h)@Whh + bh)
  h = u*h + (1-u)*c

Everything is expressed through tanh via sigmoid(z) = 0.5 + 0.5*tanh(z/2)
(r/u weight blocks pre-scaled by 0.5 on the host):
  r*h       = 0.5*h + 0.5*(tr*h)        tr = tanh(z_r/2)
  h_new     = 0.5*(h + tu*h) + (0.5 - 0.5*tu)*c
  (r*h)@Whh = h@(Whh/2) + (tr*h)@(Whh/2)   [first term off-chain]

The batch (16 rows/core) is split into two chains A, B of 8 rows run at
opposite phases; the single per-phase Tanh instruction covers chain X's
r/u gates AND chain Y's candidate in one PSUM tile, so the ACT engine
runs 2 activations per global step while both chains' latencies overlap.

All elementwise work runs on GPSIMD (cheap, SBUF-only); x-projections +
biases are matmul'd straight into the recurrence PSUM accumulators in
4-step groups.  Output h streams out in its native unit-major layout
(bf16) and is untransposed on the host.

Sharding: pure data parallel, batch 128 -> 16 rows per core x 8 cores.
"""

import sys
import numpy as np

sys.path.insert(0, "/opt/trn_rl_repo")

import concourse.bass as bass
import concourse.bacc as bacc
import concourse.tile as tile
from concourse import mybir
from concourse.masks import make_identity
from concourse.bass_utils import run_bass_kernel_spmd

B, T, IND, U = 128, 1024, 128, 256
NCORES = 8
BL = B // NCORES          # 16 batch rows per core
HB = BL // 2              # 8 rows per chain
CHUNK = 64                # timesteps per ring/output chunk
GQ = 4                    # timesteps per PSUM accumulation group
F32 = mybir.dt.float32
BF16 = mybir.dt.bfloat16

_cache = {}

# zg PSUM tile layout, [128, 192] f32 viewed (r, q, b) with r=6,q=4,b=8:
#   r=0..3: chain X's r0 r1 u0 u1 pre-acts for steps tg0+q
#   r=4..5: chain Y's c0 c1 pre-acts (phase 0: steps tg0-1+q; phase 1: tg0+q)
RU0 = 0            # col offset of ru region
C0 = 128           # col offset of c region


def _build_program():
    nc = bacc.Bacc()

    xc = nc.declare_dram_parameter("xc", [BL, T, IND], F32, isOutput=False)
    wa = nc.declare_dram_parameter("wa", [14, 128, 128], F32, isOutput=False)
    wh = nc.declare_dram_parameter("wh", [12, 128, 128], F32, isOutput=False)
    aux = nc.declare_dram_parameter("aux", [128, 192], F32, isOutput=False)
    # h layout: out[u, t*32 + ch*16 + j*8 + b] = h[ch*8+b, t, j*128+u]
    out = nc.declare_dram_parameter("out", [128, T * 32], BF16, isOutput=True)

    # projection term list for r/u: terms[g*2+j] = [(wa_idx, shift), ...]
    terms = []
    widx = 0
    for gj in range(6):
        g = gj // 2
        if g < 2:
            terms.append([(widx, 0), (widx + 1, 1), (widx + 2, 2)])
            widx += 3
        else:
            terms.append([(widx, 0)])
            widx += 1
    assert widx == 14

    Tanh = mybir.ActivationFunctionType.Tanh
    Add = mybir.AluOpType.add
    Mult = mybir.AluOpType.mult

    with tile.TileContext(nc) as tc:
        with (
            tc.tile_pool(name="singles", bufs=1) as singles,
            tc.tile_pool(name="xTp", bufs=1) as xTp,
            tc.tile_pool(name="xstage", bufs=6) as xstage,
            tc.tile_pool(name="tpsum", bufs=2, space="PSUM") as tpsum,
            tc.tile_pool(name="zg0", bufs=2, space="PSUM") as zg0p,
            tc.tile_pool(name="zg1", bufs=2, space="PSUM") as zg1p,
            tc.tile_pool(name="taup", bufs=3) as taup,
            tc.tile_pool(name="ring", bufs=3) as ringp,
            tc.tile_pool(name="small", bufs=4) as small,
        ):
            # --- resident tensors ---
            wa_f32 = singles.tile([128, 14, 128], F32)
            nc.sync.dma_start(out=wa_f32, in_=wa.rearrange("w p f -> p w f"))
            wa_sb = singles.tile([128, 14, 128], BF16)
            nc.vector.tensor_copy(wa_sb, wa_f32)
            wh_f32 = singles.tile([128, 12, 128], F32)
            nc.sync.dma_start(out=wh_f32, in_=wh.rearrange("w p f -> p w f"))
            wh_sb = singles.tile([128, 12, 128], BF16)
            nc.vector.tensor_copy(wh_sb, wh_f32)
            aux_sb = singles.tile([128, 192], F32)
            nc.sync.dma_start(out=aux_sb, in_=aux[:])
            ind6 = singles.tile([6, 192], BF16)
            nc.vector.tensor_copy(ind6, aux_sb[0:6, :])
            bias6 = singles.tile([6, 128], BF16)
            nc.vector.tensor_copy(bias6, aux_sb[32:38, 0:128])
            idt = singles.tile([128, 128], F32)
            make_identity(nc, idt)
            h0 = singles.tile([128, 32], BF16)
            nc.vector.memset(h0, 0.0)

            # --- x transposed per chain: xT[ch][p=in, col=t*8+b] bf16 ---
            xT = [xTp.tile([128, T * HB], BF16, name=f"xT{i}")
                  for i in range(2)]
            xT_v = [x.rearrange("p (t b) -> p t b", b=HB) for x in xT]

            def emit_xpose(tt, b):
                xs = xstage.tile([128, 128], F32, tag="xs")
                nc.sync.dma_start(
                    out=xs, in_=xc[b, tt * 128:(tt + 1) * 128, :])
                ps = tpsum.tile([128, 128], F32, tag="tp")
                nc.tensor.transpose(ps, xs, idt)
                nc.vector.tensor_copy(
                    xT_v[b // HB][:, tt * 128:(tt + 1) * 128, b % HB], ps)

            for b in range(BL):
                emit_xpose(0, b)

            def emit_group(ph, m):
                """Allocate + bias/x-proj for phase-ph group m.

                ru region: chain ph, steps 4m..4m+3.
                c  region: chain 1-ph, steps (4m-1..4m+2) if ph==0 else
                           (4m..4m+3).
                """
                pool = zg0p if ph == 0 else zg1p
                zg = pool.tile([128, 192], F32, tag="zg")
                nc.tensor.matmul(zg, lhsT=bias6, rhs=ind6,
                                 start=True, stop=False,
                                 skip_group_check=True)
                tg0 = 4 * m
                xr = xT[ph]
                # r/u x-projections (weights pre-scaled by 0.5)
                for gi in range(4):
                    g, j = gi // 2, gi % 2
                    for wi, shift in terms[g * 2 + j]:
                        lo = max(tg0 - shift, 0)
                        hi = min(tg0 + GQ - shift, T)
                        if lo >= hi:
                            continue
                        o = lo - (tg0 - shift)
                        nc.tensor.matmul(
                            zg[:, gi * 32 + o * 8:gi * 32 + (o + hi - lo) * 8],
                            lhsT=wa_sb[:, wi, :],
                            rhs=xr[:, lo * 8:hi * 8],
                            start=False, stop=False, skip_group_check=True)
                # c x-projections for the other chain
                tc0 = tg0 - 1 if ph == 0 else tg0
                xo = xT[1 - ph]
                lo = max(tc0, 0)
                hi = min(tc0 + GQ, T)
                o = lo - tc0
                for j in range(2):
                    wi0 = terms[4 + j][0][0]
                    nc.tensor.matmul(
                        zg[:, C0 + j * 32 + o * 8:C0 + j * 32 + (o + hi - lo) * 8],
                        lhsT=wa_sb[:, wi0, :], rhs=xo[:, lo * 8:hi * 8],
                        start=False, stop=False, skip_group_check=True)
                return zg

            def mm_ru(zg, tq, h):
                """h-matmuls for a chain's r/u gates at group slot tq."""
                for gi in range(4):
                    for k in range(2):
                        nc.tensor.matmul(
                            zg[:, gi * 32 + tq * 8:gi * 32 + tq * 8 + 8],
                            lhsT=wh_sb[:, (gi // 2) * 4 + (gi % 2) * 2 + k, :],
                            rhs=h[:, k * 8:(k + 1) * 8],
                            start=False, stop=(k == 1),
                            skip_group_check=True)

            def mm_c(zg, slot, rhs_t, final):
                """(rhs)@(Whh/2) accumulated into c region at group slot."""
                for j in range(2):
                    for k in range(2):
                        nc.tensor.matmul(
                            zg[:, C0 + j * 32 + slot * 8:C0 + j * 32 + slot * 8 + 8],
                            lhsT=wh_sb[:, 8 + j * 2 + k, :],
                            rhs=rhs_t[:, k * 8:(k + 1) * 8],
                            start=False, stop=(final and k == 1),
                            skip_group_check=True)

            # --- recurrence: chains A(=0), B(=1) at opposite phases ---
            hA = h0[:, 0:16]
            hB = h0[:, 16:32]
            ring = [None, None]
            zg = [None, None]       # current group tile per phase
            zgn = [None, None]      # pre-allocated next-group tiles
            st = {"wB": None, "sB": None, "wA": None, "sA": None}

            # prologue: groups for m=0, initial h-matmuls (h=0)
            zg[0] = emit_group(0, 0)
            zg[1] = emit_group(1, 0)
            mm_ru(zg[0], 0, hA)
            mm_c(zg[1], 0, hA, False)       # A.c(0) h-half
            mm_ru(zg[1], 0, hB)

            for k in range(T):
                tq = k % GQ
                # ---- slot T0(k): tanh over [A.ru(k) | B.c(k-1)] ----
                z0v = zg[0].rearrange("p (r q b) -> p r q b", q=GQ, b=HB)
                tau = taup.tile([128, 6, HB], F32, tag="tau")
                nc.scalar.activation(tau, z0v[:, :, tq, :], Tanh)
                tf = tau.rearrange("p r b -> p (r b)")
                # B blend for step k-1 (on-chain)
                if k > 0:
                    mB = small.tile([128, 16], BF16, tag="mB")
                    nc.gpsimd.tensor_mul(mB, st["wB"], tf[:, 32:48])
                    hnB = ring[1][:, ((k - 1) % CHUNK) * 16:
                                  ((k - 1) % CHUNK) * 16 + 16]
                    nc.gpsimd.tensor_add(hnB, mB, st["sB"])
                    hB = hnB
                    if (k - 1) % CHUNK == CHUNK - 1:
                        c0 = ((k - 1) // CHUNK) * CHUNK
                        nc.sync.dma_start(
                            out=out.rearrange("p (t c) -> p t c", c=32)
                            [:, c0:c0 + CHUNK, 16:32],
                            in_=ring[1])
                # A chain + off-chain ops: u = 0.5+0.5*tu, s = u*h,
                # w = 0.5-0.5*tu; later hnew = w*tc + s
                rhA = small.tile([128, 16], BF16, tag="rhA")
                nc.gpsimd.tensor_mul(rhA, tf[:, 0:16], hA)
                uA = small.tile([128, 16], BF16, tag="uA")
                nc.gpsimd.tensor_scalar(uA, tf[:, 16:32], 0.5, 0.5,
                                        Mult, Add)
                sA = small.tile([128, 16], BF16, tag="sA")
                nc.gpsimd.tensor_mul(sA, uA, hA)
                wA = small.tile([128, 16], BF16, tag="wA")
                nc.gpsimd.tensor_scalar(wA, tf[:, 16:32], -0.5, 0.5,
                                        Mult, Add)
                st["sA"], st["wA"] = sA, wA
                # PE: chain-critical first, then any group prealloc
                if k > 0:
                    mm_ru(zg[1], tq, hB)            # B.ru(k)
                mm_c(zg[1], tq, rhA, True)          # A.c(k) tanh-half
                if tq == 2 and k + 2 < T:
                    nm = (k + 2) // GQ
                    zgn[0] = emit_group(0, nm)
                    zgn[1] = emit_group(1, nm)
                # stream one future x-transpose every 4 steps; blocks
                # 1..7 all land well before their first use at k=128*tt
                if k % 4 == 0 and k < 448:
                    emit_xpose(k // 64 + 1, (k % 64) // 4)

                # ---- slot T1(k): tanh over [B.ru(k) | A.c(k)] ----
                z1v = zg[1].rearrange("p (r q b) -> p r q b", q=GQ, b=HB)
                tau1 = taup.tile([128, 6, HB], F32, tag="tau")
                nc.scalar.activation(tau1, z1v[:, :, tq, :], Tanh)
                tf1 = tau1.rearrange("p r b -> p (r b)")
                # A blend for step k (on-chain)
                mA = small.tile([128, 16], BF16, tag="mA")
                nc.gpsimd.tensor_mul(mA, st["wA"], tf1[:, 32:48])
                if k % CHUNK == 0:
                    ring[0] = ringp.tile([128, CHUNK * 16], BF16,
                                         tag="rgA", name="rgA")
                hnA = ring[0][:, (k % CHUNK) * 16:(k % CHUNK) * 16 + 16]
                nc.gpsimd.tensor_add(hnA, mA, st["sA"])
                hA = hnA
                if k % CHUNK == CHUNK - 1:
                    c0 = (k // CHUNK) * CHUNK
                    nc.sync.dma_start(
                        out=out.rearrange("p (t c) -> p t c", c=32)
                        [:, c0:c0 + CHUNK, 0:16],
                        in_=ring[0])
                # B chain + off-chain ops
                rhB = small.tile([128, 16], BF16, tag="rhB")
                nc.gpsimd.tensor_mul(rhB, tf1[:, 0:16], hB)
                uB = small.tile([128, 16], BF16, tag="uB")
                nc.gpsimd.tensor_scalar(uB, tf1[:, 16:32], 0.5, 0.5,
                                        Mult, Add)
                sB = small.tile([128, 16], BF16, tag="sB")
                nc.gpsimd.tensor_mul(sB, uB, hB)
                wB = small.tile([128, 16], BF16, tag="wB")
                nc.gpsimd.tensor_scalar(wB, tf1[:, 16:32], -0.5, 0.5,
                                        Mult, Add)
                st["sB"], st["wB"] = sB, wB
                if k % CHUNK == 0:
                    ring[1] = ringp.tile([128, CHUNK * 16], BF16, tag="rgB", name="rgB")
                # PE: A.MM_ru(k+1), A.c(k+1) h-half, B.c(k) both halves
                if k + 1 < T:
                    ntq = (k + 1) % GQ
                    if ntq == 0:
                        zg[0], zg[1] = zgn[0], zgn[1]
                    mm_c(zg[1], ntq, hA, False)     # A.c(k+1) h-half
                    mm_ru(zg[0], ntq, hA)           # A.ru(k+1)
                    mm_c(zg[0], ntq, hB, False)     # B.c(k) h-half
                    mm_c(zg[0], ntq, rhB, True)     # B.c(k) tanh-half
                else:
                    # epilogue group holds only B.c(T-1)
                    zg[0] = emit_group(0, T // GQ)
                    mm_c(zg[0], 0, hB, False)
                    mm_c(zg[0], 0, rhB, True)

            # ---- epilogue: tanh for B.c(T-1), final B blend ----
            z0v = zg[0].rearrange("p (r q b) -> p r q b", q=GQ, b=HB)
            tau = taup.tile([128, 6, HB], F32, tag="tau")
            nc.scalar.activation(tau, z0v[:, :, 0, :], Tanh)
            tf = tau.rearrange("p r b -> p (r b)")
            mB = small.tile([128, 16], BF16, tag="mB")
            nc.gpsimd.tensor_mul(mB, st["wB"], tf[:, 32:48])
            hnB = ring[1][:, (CHUNK - 1) * 16:CHUNK * 16]
            nc.gpsimd.tensor_add(hnB, mB, st["sB"])
            nc.sync.dma_start(
                out=out.rearrange("p (t c) -> p t c", c=32)
                [:, T - CHUNK:T, 16:32],
                in_=ring[1])
    nc.compile()
    return nc


def _fold_weights(input_weight, hidden_weight, bias, constant):
    iw = np.asarray(input_weight, np.float64)
    hw = np.asarray(hidden_weight, np.float64)
    bs = np.asarray(bias, np.float64)
    C = np.asarray(constant, np.float64)
    Wxr, Wxu, Wxh, dWxr, dWxu, ddWxr, ddWxu = [
        iw[:, i * U:(i + 1) * U] for i in range(7)]
    M = C.T @ C
    M2 = M @ M
    # r/u effective weights pre-scaled by 0.5 (sigmoid-via-tanh); the
    # c-gate hidden weight Whh is also pre-scaled by 0.5 (used twice).
    eff = {
        0: tuple(0.5 * w for w in (
            Wxr + dWxr + ddWxr, -M @ (dWxr + 2 * ddWxr), M2 @ ddWxr)),
        1: tuple(0.5 * w for w in (
            Wxu + dWxu + ddWxu, -M @ (dWxu + 2 * ddWxu), M2 @ ddWxu)),
        2: (Wxh,),
    }
    WA = np.zeros((14, 128, 128), np.float32)
    wi = 0
    for gj in range(6):
        g, j = gj // 2, gj % 2
        for Wt in eff[g]:
            WA[wi] = Wt[:, j * 128:(j + 1) * 128].astype(np.float32)
            wi += 1
    assert wi == 14
    WH = np.zeros((12, 128, 128), np.float32)
    for g in range(3):
        scale = 0.5
        Whg = scale * hw[:, g * U:(g + 1) * U]
        for j in range(2):
            for k in range(2):
                WH[g * 4 + j * 2 + k] = Whg[
                    k * 128:(k + 1) * 128, j * 128:(j + 1) * 128
                ].astype(np.float32)
    # aux: indicator (rows 0-5) + bias (rows 32-37) for the one-matmul
    # bias init of each [128,192] group tile; r/u bias pre-scaled 0.5
    AUX = np.zeros((128, 192), np.float32)
    for r in range(6):
        AUX[r, r * 32:(r + 1) * 32] = 1.0
    for gi in range(4):                      # r0 r1 u0 u1 (scaled)
        g, j = gi // 2, gi % 2
        AUX[32 + gi, 0:128] = 0.5 * bs[g * U + j * 128:g * U + (j + 1) * 128]
    for j in range(2):                       # c0 c1
        AUX[36 + j, 0:128] = bs[2 * U + j * 128:2 * U + (j + 1) * 128]
    return WA, WH, AUX


def prepare(x, input_weight, hidden_weight, bias, constant):
    x = np.ascontiguousarray(np.asarray(x, np.float32))
    WA, WH, AUX = _fold_weights(input_weight, hidden_weight, bias, constant)
    if "nc" not in _cache:
        _cache["nc"] = _build_program()
    in_maps = [
        {"xc": np.ascontiguousarray(x[i * BL:(i + 1) * BL]),
         "wa": WA, "wh": WH, "aux": AUX}
        for i in range(NCORES)
    ]
    return _cache["nc"], in_maps


def kernel(x, input_weight, hidden_weight, bias, constant):
    nc, in_maps = prepare(x, input_weight, hidden_weight, bias, constant)
    last_err = None
    for _ in range(3):          # retry transient device errors
        try:
            res = run_bass_kernel_spmd(nc, in_maps, list(range(NCORES)))
            outs = []
            for i in range(NCORES):
                o = np.asarray(res.results[i]["out"]).astype(np.float32)
                # out[u, t*32 + ch*16 + j*8 + b] = h[ch*8+b, t, j*128+u]
                outs.append(
                    o.reshape(128, T, 2, 2, HB).transpose(2, 4, 1, 3, 0)
                    .reshape(BL, T, U))
            return np.concatenate(outs, axis=0)
        except Exception as e:   # e.g. NRT_EXEC_UNIT_UNRECOVERABLE
            last_err = e
    raise last_err


if __name__ == "__main__":
    rng = np.random.default_rng(0)
    x = rng.standard_normal((B, T, IND), dtype=np.float32)
    iw = (rng.standard_normal((IND, 7 * U)) * 0.05).astype(np.float32)
    hw = (rng.standard_normal((U, 3 * U)) * 0.05).astype(np.float32)
    bs = np.zeros(3 * U, np.float32)
    C = np.concatenate([np.eye(IND, dtype=np.float32),
                        np.zeros((U - IND, IND), np.float32)], 0)
    y = kernel(x, iw, hw, bs, C)
    print("out", y.shape, y.dtype, float(np.abs(y).mean()))


# revision 20
# speedup vs baseline: 1.0248x; 1.0248x over previous
"""Trainium2 Bass kernel for nn_AnotherDDoIGRUCell.

Math (per timestep, parallel part folded on host):
  x_tm1 = x_{t-1} @ M,  x_tm2 = x_{t-2} @ M @ M   with M = C.T @ C
  d_x   = x_t - x_tm1 ; dd_x = x_t - 2*x_tm1 + x_tm2
  r = sigmoid(x_t@Wxr + d_x@dWxr + dd_x@ddWxr + h@Whr + br)
  u = sigmoid(x_t@Wxu + d_x@dWxu + dd_x@ddWxu + h@Whu + bu)
  c = tanh   (x_t@Wxh + (r*h)@Whh + bh)
  h = u*h + (1-u)*c

The x-projections collapse to 3 effective weights per gate applied to
x_t, x_{t-1}, x_{t-2}. They are computed (together with the bias, via a
tiny indicator matmul) straight into the recurrence PSUM accumulators in
4-step groups, so no separate pre-activation buffer or copies exist.

Per-step critical chain: MM(ru) -> sigmoid(r,u merged, PSUM->PSUM) ->
r*h (Pool) -> MM(c) -> tanh (PSUM->PSUM) -> m2,hnew (Pool).  All
elementwise work runs on GPSIMD; ACT only does the two activations.
Output h is DMA'd out in its native transposed (unit-major) layout and
untransposed on the host.

Sharding: pure data parallel, batch 128 -> 16 rows per core x 8 cores.
"""

import sys
import numpy as np

sys.path.insert(0, "/opt/trn_rl_repo")

import concourse.bass as bass
import concourse.bacc as bacc
import concourse.tile as tile
from concourse import mybir
from concourse.masks import make_identity
from concourse.bass_utils import run_bass_kernel_spmd

B, T, IND, U = 128, 1024, 128, 256
NCORES = 8
BL = B // NCORES          # 16 batch rows per core
CHUNK = 64                # timesteps per ring/output chunk
GQ = 4                    # timesteps per PSUM accumulation group
F32 = mybir.dt.float32
BF16 = mybir.dt.bfloat16

_cache = {}


def _build_program():
    nc = bacc.Bacc()

    xc = nc.declare_dram_parameter("xc", [BL, T, IND], F32, isOutput=False)
    wa = nc.declare_dram_parameter("wa", [14, 128, 128], F32, isOutput=False)
    wh = nc.declare_dram_parameter("wh", [12, 128, 128], F32, isOutput=False)
    aux = nc.declare_dram_parameter("aux", [128, 4 * GQ * 16], F32, isOutput=False)
    # h in on-chip layout: out[u_low, t*32 + j*16 + b] = h[b, t, j*128+u_low]
    out = nc.declare_dram_parameter("out", [128, T * 2 * BL], BF16, isOutput=True)

    # projection term list: terms[gj] = [(wa_idx, shift), ...]
    terms = []
    widx = 0
    for gj in range(6):
        g = gj // 2
        if g < 2:
            terms.append([(widx, 0), (widx + 1, 16), (widx + 2, 32)])
            widx += 3
        else:
            terms.append([(widx, 0)])
            widx += 1
    assert widx == 14

    Sig = mybir.ActivationFunctionType.Sigmoid
    Tanh = mybir.ActivationFunctionType.Tanh
    Add = mybir.AluOpType.add
    Mult = mybir.AluOpType.mult

    with tile.TileContext(nc) as tc:
        with (
            tc.tile_pool(name="singles", bufs=1) as singles,
            tc.tile_pool(name="xT", bufs=1) as xT_pool,
            tc.tile_pool(name="xstage", bufs=6) as xstage,
            tc.tile_pool(name="tpsum", bufs=2, space="PSUM") as tpsum,
            tc.tile_pool(name="gps4", bufs=2, space="PSUM") as gps4,
            tc.tile_pool(name="gps4c", bufs=2, space="PSUM") as gps4c,
            tc.tile_pool(name="taup", bufs=3) as taup,
            tc.tile_pool(name="ring", bufs=3) as ringp,
            tc.tile_pool(name="small", bufs=3) as small,
        ):
            # --- resident tensors ---
            wa_f32 = singles.tile([128, 14, 128], F32)
            nc.sync.dma_start(out=wa_f32, in_=wa.rearrange("w p f -> p w f"))
            wa_sb = singles.tile([128, 14, 128], BF16)
            nc.vector.tensor_copy(wa_sb, wa_f32)
            wh_f32 = singles.tile([128, 12, 128], F32)
            nc.sync.dma_start(out=wh_f32, in_=wh.rearrange("w p f -> p w f"))
            wh_sb = singles.tile([128, 12, 128], BF16)
            nc.vector.tensor_copy(wh_sb, wh_f32)
            aux_sb = singles.tile([128, 4 * GQ * 16], F32)
            nc.sync.dma_start(out=aux_sb, in_=aux[:])
            ind_ru = singles.tile([4, 4 * GQ * 16], BF16)
            nc.vector.tensor_copy(ind_ru, aux_sb[0:4, :])
            ind_c = singles.tile([2, 2 * GQ * 16], BF16)
            nc.vector.tensor_copy(ind_c, aux_sb[32:34, 0:2 * GQ * 16])
            bias_ru = singles.tile([4, 128], BF16)
            nc.vector.tensor_copy(bias_ru, aux_sb[64:68, 0:128])
            bias_c = singles.tile([2, 128], BF16)
            nc.vector.tensor_copy(bias_c, aux_sb[96:98, 0:128])
            idt = singles.tile([128, 128], F32)
            make_identity(nc, idt)
            h0 = singles.tile([128, 32], BF16)
            nc.vector.memset(h0, 0.0)

            # --- transpose x into xT[p=in_dim, col=t*16+b] (bf16) ---
            # Only tt=0 (t<128, covering chunks 0-1) is transposed up
            # front; the remaining tt blocks stream into the chunk loop
            # so the recurrence starts ~10us in instead of ~40us.
            xT = xT_pool.tile([128, T * BL], BF16)
            xT_v = xT.rearrange("p (t b) -> p t b", b=BL)
            dma_engines = [nc.sync, nc.scalar]

            def emit_xpose(tt, b):
                xs = xstage.tile([128, 128], F32, tag="xs")
                dma_engines[(tt * BL + b) % 2].dma_start(
                    out=xs, in_=xc[b, tt * 128:(tt + 1) * 128, :])
                ps = tpsum.tile([128, 128], F32, tag="tp")
                nc.tensor.transpose(ps, xs, idt)
                nc.vector.tensor_copy(
                    xT_v[:, tt * 128:(tt + 1) * 128, b], ps)

            for b in range(BL):
                emit_xpose(0, b)

            # --- recurrence ---
            # group PSUM layouts:
            #   prug [128, 256]: col = gi*64 + tq*16 + b   (gi: r0 r1 u0 u1)
            #   pcg  [128, 128]: col = j*64 + tq*16 + b    (j: c0 c1)
            # Projections are emitted in two halves, two steps before the
            # group is first consumed, so the PE bursts never queue ahead
            # of chain-critical matmuls.
            def proj_a(prug, tg0):
                """bias + r-gate x-projections for steps [tg0, tg0+GQ)"""
                gc0 = tg0 * 16
                ncols = GQ * 16
                nc.tensor.matmul(prug, lhsT=bias_ru, rhs=ind_ru,
                                 start=True, stop=False,
                                 skip_group_check=True)
                for gi in range(2):
                    for wi, shift in terms[gi]:
                        o = max(0, shift - gc0)
                        nc.tensor.matmul(
                            prug[:, gi * ncols + o:(gi + 1) * ncols],
                            lhsT=wa_sb[:, wi, :],
                            rhs=xT[:, gc0 + o - shift:gc0 + ncols - shift],
                            start=False, stop=False, skip_group_check=True)

            def proj_b(prug, pcg, tg0):
                """u-gate + candidate x-projections"""
                gc0 = tg0 * 16
                ncols = GQ * 16
                nc.tensor.matmul(pcg, lhsT=bias_c, rhs=ind_c,
                                 start=True, stop=False,
                                 skip_group_check=True)
                for gi in range(2, 4):
                    for wi, shift in terms[gi]:
                        o = max(0, shift - gc0)
                        nc.tensor.matmul(
                            prug[:, gi * ncols + o:(gi + 1) * ncols],
                            lhsT=wa_sb[:, wi, :],
                            rhs=xT[:, gc0 + o - shift:gc0 + ncols - shift],
                            start=False, stop=False, skip_group_check=True)
                for j in range(2):
                    wi0 = terms[4 + j][0][0]
                    nc.tensor.matmul(
                        pcg[:, j * ncols:(j + 1) * ncols],
                        lhsT=wa_sb[:, wi0, :], rhs=xT[:, gc0:gc0 + ncols],
                        start=False, stop=False, skip_group_check=True)

            def mm_ru(prug, q0, rhs_t, final):
                """rhs@W_h(r,u) accumulated into the step's psum cols.
                h(k-1) = s(k-1) + m2(k-1) is fed as two separate rhs
                tensors so the s-part runs before the chain needs it."""
                for gi in range(4):          # r0 r1 u0 u1
                    for kk in range(2):
                        nc.tensor.matmul(
                            prug[:, gi * GQ * 16 + q0:gi * GQ * 16 + q0 + 16],
                            lhsT=wh_sb[:, (gi // 2) * 4 + (gi % 2) * 2 + kk, :],
                            rhs=rhs_t[:, kk * 16:(kk + 1) * 16],
                            start=False, stop=(final and gi == 3 and kk == 1),
                            skip_group_check=True)

            hT = h0
            sPrev = m2Prev = None
            # group 0 projections up front
            prugN = gps4.tile([128, 4 * GQ * 16], F32, tag="g4", name="g4")
            pcgN = gps4c.tile([128, 2 * GQ * 16], F32, tag="g4c", name="g4c")
            proj_a(prugN, 0)
            proj_b(prugN, pcgN, 0)
            prug = pcg = None
            for chunk in range(T // CHUNK):
                t0 = chunk * CHUNK
                ring = ringp.tile([128, CHUNK * 32], BF16, tag="ring")
                for tl in range(CHUNK):
                    tq = tl % GQ
                    k = t0 + tl
                    if tq == 0:
                        prug, pcg = prugN, pcgN
                        # stream one future x-transpose per group: during
                        # even chunk c, transpose block tt = c//2 + 1
                        if chunk % 2 == 0 and chunk // 2 + 1 < T // 128:
                            emit_xpose(chunk // 2 + 1, tl // GQ)
                    q0 = tq * 16
                    # r/u h-matmuls: s-part early, m2-part is the only
                    # chain-critical input
                    if k > 0:
                        mm_ru(prug, q0, sPrev, False)
                        mm_ru(prug, q0, m2Prev, True)
                    pr_v = prug.rearrange("p (g x) -> p g x", g=4)
                    tau = taup.tile([128, 6, 16], F32, tag="tau")
                    tau_f = tau.rearrange("p g x -> p (g x)")
                    # r gate alone on the chain; u follows on ACT
                    nc.scalar.activation(
                        tau[:, 0:2, :], pr_v[:, 0:2, q0:q0 + 16], Sig)
                    nc.scalar.activation(
                        tau[:, 2:4, :], pr_v[:, 2:4, q0:q0 + 16], Sig)
                    rh = small.tile([128, 32], BF16, tag="rh")
                    nc.gpsimd.tensor_mul(rh, tau_f[:, 0:32], hT)
                    # candidate h-matmuls
                    for j in range(2):
                        for kk in range(2):
                            nc.tensor.matmul(
                                pcg[:, j * GQ * 16 + q0:j * GQ * 16 + q0 + 16],
                                lhsT=wh_sb[:, 8 + j * 2 + kk, :],
                                rhs=rh[:, kk * 16:(kk + 1) * 16],
                                start=False, stop=(kk == 1),
                                skip_group_check=True)
                    # u path on Pool (s = u*h, uc = 1-u), off the chain
                    s = small.tile([128, 32], BF16, tag="s")
                    nc.gpsimd.tensor_mul(s, tau_f[:, 32:64], hT)
                    uc = small.tile([128, 32], BF16, tag="uc")
                    nc.gpsimd.tensor_scalar(
                        uc, tau_f[:, 32:64], -1.0, 1.0, Mult, Add)
                    # next group's projections, split over tq==2 / tq==3
                    if tq == 2 and k + 2 < T:
                        prugN = gps4.tile([128, 4 * GQ * 16], F32,
                                          tag="g4", name="g4")
                        pcgN = gps4c.tile([128, 2 * GQ * 16], F32,
                                          tag="g4c", name="g4c")
                        proj_a(prugN, k + 2)
                    elif tq == 3 and k + 1 < T:
                        proj_b(prugN, pcgN, k + 1)
                    pc_v = pcg.rearrange("p (g x) -> p g x", g=2)
                    nc.scalar.activation(
                        tau[:, 4:6, :], pc_v[:, :, q0:q0 + 16], Tanh)
                    # blend: m2 = (1-u)*c on the chain only as matmul
                    # input; h itself is materialized off-chain
                    m2 = small.tile([128, 32], BF16, tag="m2")
                    nc.gpsimd.tensor_mul(m2, uc, tau_f[:, 64:96])
                    hnew = ring[:, tl * 32:(tl + 1) * 32]
                    nc.gpsimd.tensor_add(hnew, m2, s)
                    hT = hnew
                    sPrev, m2Prev = s, m2

                # chunk done: stream the ring straight out (bf16,
                # unit-major); host untransposes
                nc.sync.dma_start(
                    out=out[:, t0 * 32:(t0 + CHUNK) * 32], in_=ring)
    nc.compile()
    return nc


def _fold_weights(input_weight, hidden_weight, bias, constant):
    iw = np.asarray(input_weight, np.float64)
    hw = np.asarray(hidden_weight, np.float64)
    bs = np.asarray(bias, np.float64)
    C = np.asarray(constant, np.float64)
    Wxr, Wxu, Wxh, dWxr, dWxu, ddWxr, ddWxu = [
        iw[:, i * U:(i + 1) * U] for i in range(7)]
    M = C.T @ C
    M2 = M @ M
    eff = {
        0: (Wxr + dWxr + ddWxr, -M @ (dWxr + 2 * ddWxr), M2 @ ddWxr),
        1: (Wxu + dWxu + ddWxu, -M @ (dWxu + 2 * ddWxu), M2 @ ddWxu),
        2: (Wxh,),
    }
    WA = np.zeros((14, 128, 128), np.float32)
    wi = 0
    for gj in range(6):
        g, j = gj // 2, gj % 2
        for Wt in eff[g]:
            WA[wi] = Wt[:, j * 128:(j + 1) * 128].astype(np.float32)
            wi += 1
    assert wi == 14
    WH = np.zeros((12, 128, 128), np.float32)
    for g in range(3):
        Whg = hw[:, g * U:(g + 1) * U]
        for j in range(2):
            for k in range(2):
                WH[g * 4 + j * 2 + k] = Whg[
                    k * 128:(k + 1) * 128, j * 128:(j + 1) * 128
                ].astype(np.float32)
    # aux: indicator + bias rows (32-aligned) for in-psum bias matmuls
    blk = GQ * 16
    AUX = np.zeros((128, 4 * blk), np.float32)
    for k in range(4):                       # ind_ru[k, c] = (c//blk == k)
        AUX[k, k * blk:(k + 1) * blk] = 1.0
    for k in range(2):                       # ind_c[k, c] = (c//blk == k)
        AUX[32 + k, k * blk:(k + 1) * blk] = 1.0
    for gi in range(4):                      # bias r0 r1 u0 u1
        g, j = gi // 2, gi % 2
        AUX[64 + gi, 0:128] = bs[g * U + j * 128:g * U + (j + 1) * 128]
    for j in range(2):                       # bias c0 c1
        AUX[96 + j, 0:128] = bs[2 * U + j * 128:2 * U + (j + 1) * 128]
    return WA, WH, AUX


def prepare(x, input_weight, hidden_weight, bias, constant):
    x = np.ascontiguousarray(np.asarray(x, np.float32))
    WA, WH, AUX = _fold_weights(input_weight, hidden_weight, bias, constant)
    if "nc" not in _cache:
        _cache["nc"] = _build_program()
    in_maps = [
        {"xc": np.ascontiguousarray(x[i * BL:(i + 1) * BL]),
         "wa": WA, "wh": WH, "aux": AUX}
        for i in range(NCORES)
    ]
    return _cache["nc"], in_maps


def kernel(x, input_weight, hidden_weight, bias, constant):
    nc, in_maps = prepare(x, input_weight, hidden_weight, bias, constant)
    last_err = None
    for _ in range(3):          # retry transient device errors
        try:
            res = run_bass_kernel_spmd(nc, in_maps, list(range(NCORES)))
            outs = []
            for i in range(NCORES):
                o = np.asarray(res.results[i]["out"]).astype(np.float32)
                # out[u, t*32 + j*16 + b] = h[b, t, j*128 + u]
                outs.append(
                    o.reshape(128, T, 2, BL).transpose(3, 1, 2, 0)
                    .reshape(BL, T, U))
            return np.concatenate(outs, axis=0)
        except Exception as e:   # e.g. NRT_EXEC_UNIT_UNRECOVERABLE
            last_err = e
    raise last_err


if __name__ == "__main__":
    rng = np.random.default_rng(0)
    x = rng.standard_normal((B, T, IND), dtype=np.float32)
    iw = (rng.standard_normal((IND, 7 * U)) * 0.05).astype(np.float32)
    hw = (rng.standard_normal((U, 3 * U)) * 0.05).astype(np.float32)
    bs = np.zeros(3 * U, np.float32)
    C = np.concatenate([np.eye(IND, dtype=np.float32),
                        np.zeros((U - IND, IND), np.float32)], 0)
    y = kernel(x, iw, hw, bs, C)
    print("out", y.shape, y.dtype, float(np.abs(y).mean()))
